# revision 13
# baseline (speedup 1.0000x reference)
"""Two-layer GAT (PyG GATConv x2) on 8 Trainium2 NeuronCores via Bass.

Strategy (dst-sharded, graph-parallel):
- Nodes sharded 8 ways by destination range (6250/core, padded to 6272).
- Per layer: local feature matmul -> build a gather table row per node
  [G-per-head|1.0 ... | al_src | al_dst] in bf16 -> on-device AllGather ->
  edge phase: edges sorted by dst window (128 dsts), bulk dma_gather of
  source rows, softmax WITHOUT max-subtraction (exponents bounded, fp32-safe),
  segment-sums via one-hot "staircase" mask matmuls accumulating in PSUM.
  The softmax denominator rides in the same matmul through baked 1.0 columns.
  al_dst is broadcast dst->edges with a PE transpose of the mask + tiny matmul
  (avoids a second per-edge gather).
- leaky_relu inside exp via exp(leaky(t)) = max(exp(t), exp(0.2 t)).
"""
import math
import sys

import numpy as np
import ml_dtypes

sys.path.insert(0, '/opt/trn_rl_repo')

bf16 = ml_dtypes.bfloat16

P = 128
NCORE = 8
N = 50000
NSH = 6250
NSHP = 6272          # 49 * 128
NW = NSHP // P       # 49 windows
HALF = 4 * NSHP      # 25088 rows per half-table
CIN = 256
H = 4
D1 = 64
D2 = 32
R1 = 384                  # table-1 row stride (256B-mult; content 268)
R2 = 256                  # table-2 row stride (256B-mult; content 140)
ES1 = 384                 # gather elem count L1 (768B, covers 268 + spill)
ES2 = 256                 # gather elem count L2 (512B, covers 140 + spill)
TROWS = 2 * HALF + 256    # table alloc rows (incl. spill pad)
PADREL = 200.0            # dstrel sentinel for pad edges (kills mask column)
NEG = 0.2


def _prep(edge_index):
    """Host-side: shard + sort edges, build schedule and index arrays."""
    src = np.concatenate([edge_index[0], np.arange(N, dtype=np.int64)]).astype(np.int64)
    dst = np.concatenate([edge_index[1], np.arange(N, dtype=np.int64)]).astype(np.int64)
    owner = dst // NSH
    dloc = (dst - owner * NSH).astype(np.int32)
    srcpad = ((src // NSH) * NSHP + (src % NSH)).astype(np.int32)
    w = dloc // P
    drel = (dloc % P).astype(np.int32)
    half = (srcpad >= HALF).astype(np.int32)
    srcrel = np.where(half == 1, srcpad - HALF, srcpad).astype(np.int32)

    # per (core, window, half) edge lists
    counts = np.zeros((NCORE, NW, 2), np.int64)
    percore = []
    for k in range(NCORE):
        sel = np.nonzero(owner == k)[0]
        key = (w[sel] * 2 + half[sel]).astype(np.int64)
        order = np.argsort(key, kind='stable')
        sel = sel[order]
        kk = key[order]
        cnt = np.bincount(kk, minlength=NW * 2).reshape(NW, 2)
        counts[k] = cnt
        percore.append((srcrel[sel], drel[sel], cnt))

    nblk = np.maximum(1, np.ceil(counts.max(axis=0) / P).astype(np.int64))  # [NW,2]
    block_meta = []           # (window, half) per block
    for wi in range(NW):
        for h in range(2):
            block_meta += [(wi, h)] * int(nblk[wi, h])
    totblk = len(block_meta)

    # calls: runs of consecutive same-half blocks, <= 8 blocks each
    calls = []                # (b0, nb, half, col0)
    col = 0
    b = 0
    while b < totblk:
        h = block_meta[b][1]
        nb = 1
        while (b + nb < totblk and block_meta[b + nb][1] == h
               and nb < 8):
            nb += 1
        calls.append((b, nb, h, col))
        col += nb * 8
        b += nb
    ccols = col

    # per-core lane arrays
    srcidx = np.zeros((NCORE, totblk, P), np.int16)
    dstrel = np.full((NCORE, totblk, P), PADREL, np.float32)
    for k in range(NCORE):
        es, ed, cnt = percore[k]
        pos = 0
        blk = 0
        for wi in range(NW):
            for h in range(2):
                c = int(cnt[wi, h])
                nb = int(nblk[wi, h])
                lanes = np.arange(c)
                srcidx[k, blk + lanes // P, lanes % P] = es[pos:pos + c]
                dstrel[k, blk + lanes // P, lanes % P] = ed[pos:pos + c]
                pos += c
                blk += nb
        assert pos == len(es)

    # pack call indices: [128, ccols] int16 per core
    srcpk = np.zeros((NCORE, P, ccols), np.int16)
    for k in range(NCORE):
        for (b0, nb, h, c0) in calls:
            ni = nb * P
            flat = srcidx[k, b0:b0 + nb].reshape(ni)   # flat[j*128+p]
            wrap = flat.reshape(-1, 16).T              # [16, ni/16]
            srcpk[k, :, c0:c0 + ni // 16] = np.tile(wrap, (8, 1))
    dstrel_pk = np.ascontiguousarray(
        dstrel.transpose(0, 2, 1)).astype(np.float32)  # [NCORE, 128, totblk]

    first_of_w = {}
    last_of_w = {}
    for b, (wi, h) in enumerate(block_meta):
        if wi not in first_of_w:
            first_of_w[wi] = b
        last_of_w[wi] = b
    return dict(block_meta=block_meta, calls=calls, totblk=totblk, ccols=ccols,
                srcpk=srcpk, dstrel_pk=dstrel_pk,
                first_of_w=first_of_w, last_of_w=last_of_w)


def _build(sched):
    import concourse.bass as bass
    import concourse.tile as tile
    from concourse import bacc, mybir, library_config
    from concourse.bass import AP

    dt = mybir.dt
    Alu = mybir.AluOpType
    Act = mybir.ActivationFunctionType

    totblk = sched['totblk']
    ccols = sched['ccols']
    calls = sched['calls']
    block_meta = sched['block_meta']
    first_of_w = sched['first_of_w']
    last_of_w = sched['last_of_w']

    nc = bacc.Bacc("TRN2", target_bir_lowering=False, debug=False,
                   num_devices=NCORE, num_swdge_queues=4)

    # ---- I/O ----
    xT = nc.dram_tensor("xT", [CIN, NSHP], dt.bfloat16, kind="ExternalInput")
    W1 = nc.dram_tensor("W1b", [CIN, CIN], dt.bfloat16, kind="ExternalInput")
    W2 = nc.dram_tensor("W2b", [CIN, H * D2], dt.bfloat16, kind="ExternalInput")
    a1r = nc.dram_tensor("a1r", [P, 2 * CIN], dt.float32, kind="ExternalInput")
    a2r = nc.dram_tensor("a2r", [P, 2 * H * D2], dt.float32, kind="ExternalInput")
    b1r = nc.dram_tensor("b1r", [P, CIN], dt.float32, kind="ExternalInput")
    b2r = nc.dram_tensor("b2r", [P, D2], dt.float32, kind="ExternalInput")
    iota_in = nc.dram_tensor("iota_in", [P, P], dt.float32, kind="ExternalInput")
    ident_in = nc.dram_tensor("ident_in", [P, P], dt.bfloat16, kind="ExternalInput")
    sidx = nc.dram_tensor("sidx", [P, ccols], dt.int16, kind="ExternalInput")
    drel = nc.dram_tensor("drel", [P, totblk], dt.float32, kind="ExternalInput")
    out_t = nc.dram_tensor("out", [NSHP, D2], dt.float32, kind="ExternalOutput")

    # ---- internal DRAM ----
    t1own = nc.dram_tensor("t1own", [NSHP, R1], dt.bfloat16)
    t2own = nc.dram_tensor("t2own", [NSHP, R2], dt.bfloat16)
    T1 = nc.dram_tensor("T1", [TROWS, R1], dt.bfloat16, addr_space="Shared")
    T2 = nc.dram_tensor("T2", [TROWS, R2], dt.bfloat16, addr_space="Shared")

    rg = [list(range(NCORE))]

    with tile.TileContext(nc) as tc:
        import contextlib
        ctx = contextlib.ExitStack()
        with ctx:
            cpool = ctx.enter_context(tc.tile_pool(name="consts", bufs=1))
            gpool = ctx.enter_context(tc.tile_pool(name="g", bufs=10))
            gspool = ctx.enter_context(tc.tile_pool(name="gs", bufs=6))
            mpool = ctx.enter_context(tc.tile_pool(name="mask", bufs=4))
            spool = ctx.enter_context(tc.tile_pool(name="ssb", bufs=8))
            epool = ctx.enter_context(tc.tile_pool(name="ex", bufs=12))
            wpool = ctx.enter_context(tc.tile_pool(name="wend", bufs=6))
            rowpool = ctx.enter_context(tc.tile_pool(name="trow", bufs=4))
            xpool = ctx.enter_context(tc.tile_pool(name="xt", bufs=4))
            pw_pool = ctx.enter_context(tc.tile_pool(name="pw", bufs=3, space="PSUM"))
            tps_pool = ctx.enter_context(tc.tile_pool(name="tps", bufs=2, space="PSUM"))
            ad_pool = ctx.enter_context(tc.tile_pool(name="adp", bufs=2, space="PSUM"))
            hp_pool = ctx.enter_context(tc.tile_pool(name="hps", bufs=1, space="PSUM"))

            nc.gpsimd.load_library(library_config.mlp)

            # ---- persistent constants ----
            iota_sb = cpool.tile([P, P], dt.float32, tag="iota")
            nc.sync.dma_start(out=iota_sb[:], in_=iota_in[:, :])
            ident_sb = cpool.tile([P, P], dt.bfloat16, tag="ident")
            nc.sync.dma_start(out=ident_sb[:], in_=ident_in[:, :])
            w1a = cpool.tile([P, CIN], dt.bfloat16, tag="w1a")
            nc.sync.dma_start(out=w1a[:], in_=W1[0:P, :])
            w1b = cpool.tile([P, CIN], dt.bfloat16, tag="w1b")
            nc.sync.dma_start(out=w1b[:], in_=W1[P:2 * P, :])
            w2a = cpool.tile([P, H * D2], dt.bfloat16, tag="w2a")
            nc.sync.dma_start(out=w2a[:], in_=W2[0:P, :])
            w2b = cpool.tile([P, H * D2], dt.bfloat16, tag="w2b")
            nc.sync.dma_start(out=w2b[:], in_=W2[P:2 * P, :])
            a1_sb = cpool.tile([P, 2 * CIN], dt.float32, tag="a1")
            nc.sync.dma_start(out=a1_sb[:], in_=a1r[:, :])
            a2_sb = cpool.tile([P, 2 * H * D2], dt.float32, tag="a2")
            nc.sync.dma_start(out=a2_sb[:], in_=a2r[:, :])
            b1_sb = cpool.tile([P, CIN], dt.float32, tag="b1")
            nc.sync.dma_start(out=b1_sb[:], in_=b1r[:, :])
            b2_sb = cpool.tile([P, D2], dt.float32, tag="b2")
            nc.sync.dma_start(out=b2_sb[:], in_=b2r[:, :])
            sidx_sb = cpool.tile([P, ccols], dt.int16, tag="sidx")
            nc.sync.dma_start(out=sidx_sb[:], in_=sidx[:, :])
            drel_sb = cpool.tile([P, totblk], dt.float32, tag="drel")
            nc.sync.dma_start(out=drel_sb[:], in_=drel[:, :])
            alown1 = cpool.tile([P, NW * 2 * H], dt.bfloat16, tag="alo1")
            alown2 = cpool.tile([P, NW * 2 * H], dt.bfloat16, tag="alo2")
            hp_sb = cpool.tile([P, NW * CIN], dt.bfloat16, tag="hp")

            def node_matmul_phase(layer):
                """x@W (or h'@W2) per window; build table rows; als."""
                for wi in range(NW):
                    if layer == 1:
                        ps = hp_pool.tile([P, CIN], dt.float32, space="PSUM", tag="hps")
                        la = xpool.tile([P, P], dt.bfloat16, tag="xa")
                        nc.sync.dma_start(out=la[:], in_=xT[0:P, wi * P:(wi + 1) * P])
                        lb = xpool.tile([P, P], dt.bfloat16, tag="xb")
                        nc.sync.dma_start(out=lb[:], in_=xT[P:2 * P, wi * P:(wi + 1) * P])
                        nc.tensor.matmul(out=ps[:], lhsT=la[:], rhs=w1a[:],
                                         start=True, stop=False)
                        nc.tensor.matmul(out=ps[:], lhsT=lb[:], rhs=w1b[:],
                                         start=False, stop=True)
                        width, dg, rw, es, alo, a_sb = CIN, D1, R1, ES1, alown1, a1_sb
                        town = t1own
                    else:
                        # transpose h' window chunks -> lhsT
                        hsl = hp_sb[:, wi * CIN:(wi + 1) * CIN]
                        ps = hp_pool.tile([P, H * D2], dt.float32, space="PSUM", tag="hps")
                        for kc in range(2):
                            tp = tps_pool.tile([P, P], dt.bfloat16, space="PSUM", tag="tps")
                            nc.tensor.transpose(out=tp[:], in_=hsl[:, kc * P:(kc + 1) * P],
                                                identity=ident_sb[:])
                            hT = spool.tile([P, P], dt.bfloat16, tag="hT")
                            nc.scalar.copy(out=hT[:], in_=tp[:])
                            nc.tensor.matmul(out=ps[:], lhsT=hT[:],
                                             rhs=(w2a if kc == 0 else w2b)[:],
                                             start=(kc == 0), stop=(kc == 1))
                        width, dg, rw, es, alo, a_sb = H * D2, D2, R2, ES2, alown2, a2_sb
                        town = t2own

                    # als: tmp = ps(x2) * a_rep ; reduce per head
                    tmp = wpool.tile([P, 2 * width], dt.float32, tag="altmp")
                    nc.vector.tensor_tensor(
                        out=tmp[:], in0=ps[:].unsqueeze(1).to_broadcast([P, 2, width]),
                        in1=a_sb[:].rearrange("p (t x) -> p t x", t=2),
                        op=Alu.mult)
                    alof = wpool.tile([P, 8], dt.float32, tag="alof")
                    nc.vector.tensor_reduce(
                        out=alof[:],
                        in_=tmp[:].rearrange("p (t h d) -> p t h d", t=2, h=H),
                        axis=mybir.AxisListType.X, op=Alu.add)
                    nc.vector.tensor_copy(out=alo[:, wi * 8:wi * 8 + 8], in_=alof[:])

                    # table row
                    tr = rowpool.tile([P, rw], dt.bfloat16, tag="trow")
                    nc.gpsimd.memset(tr[:], 1.0)
                    nc.scalar.copy(
                        out=tr[:, 0:H * (dg + 2)].rearrange(
                            "p (h y) -> p h y", h=H)[:, :, 0:dg],
                        in_=ps[:].rearrange("p (h d) -> p h d", h=H))
                    nc.vector.tensor_copy(
                        out=tr[:, H * (dg + 2):H * (dg + 2) + 8], in_=alof[:])
                    nc.sync.dma_start(out=town[wi * P:(wi + 1) * P, :], in_=tr[:])

            def edge_phase(layer):
                T = T1 if layer == 1 else T2
                rw = R1 if layer == 1 else R2
                es = ES1 if layer == 1 else ES2
                dg = D1 if layer == 1 else D2
                width = H * (dg + 2)
                alo = alown1 if layer == 1 else alown2
                pw = {}
                masks = {}
                qn = 0
                for ci, (b0, nb, hf, c0) in enumerate(calls):
                    ni = nb * P
                    g = gpool.tile([P, 8 * es], dt.bfloat16, tag="g")
                    in_ap = AP(T, hf * HALF * rw, [[rw, TROWS - hf * HALF], [1, es]])
                    nc.gpsimd.dma_gather(
                        g[:, 0:nb * es].rearrange("p (b w) -> p b w", w=es),
                        in_ap, sidx_sb[:, c0:c0 + ni // 16], ni, ni, es,
                        elem_step=rw, queue_num=qn)
                    qn = (qn + 1) % 4

                    # masks (one grouped op per call) + Ad broadcast
                    adps = ad_pool.tile([P, 8 * H], dt.float32, space="PSUM", tag="adp")
                    mt = mpool.tile([P, 8 * P], dt.bfloat16, tag="m")
                    nc.vector.tensor_tensor(
                        out=mt[:, 0:nb * P].rearrange("p (b q) -> p b q", q=P),
                        in0=iota_sb[:, 0:P].unsqueeze(1).to_broadcast([P, nb, P]),
                        in1=drel_sb[:, b0:b0 + nb].unsqueeze(2).to_broadcast(
                            [P, nb, P]),
                        op=Alu.is_equal)
                    for j in range(nb):
                        b = b0 + j
                        wi = block_meta[b][0]
                        masks[b] = mt[:, j * P:(j + 1) * P]
                        tp = tps_pool.tile([P, P], dt.bfloat16, space="PSUM", tag="tps")
                        nc.tensor.transpose(out=tp[:], in_=masks[b],
                                            identity=ident_sb[:])
                        ssb = spool.tile([P, P], dt.bfloat16, tag="s")
                        nc.scalar.copy(out=ssb[:], in_=tp[:])
                        nc.tensor.matmul(
                            out=adps[:, j * H:(j + 1) * H], lhsT=ssb[:],
                            rhs=alo[:, wi * 8 + 4:wi * 8 + 8],
                            start=True, stop=True)

                    # ex chain (batched over nb blocks)
                    t = epool.tile([P, 8 * H], dt.float32, tag="t")
                    nc.vector.tensor_tensor(
                        out=t[:, 0:nb * H],
                        in0=g[:, 0:nb * es].rearrange(
                            "p (b w) -> p b w", w=es)[:, :, width:width + H],
                        in1=adps[:, 0:nb * H].rearrange("p (b h) -> p b h", h=H),
                        op=Alu.add)
                    e1 = epool.tile([P, 8 * H], dt.float32, tag="e1")
                    nc.scalar.activation(out=e1[:, 0:nb * H], in_=t[:, 0:nb * H],
                                         func=Act.Exp)
                    e2 = epool.tile([P, 8 * H], dt.float32, tag="e2")
                    nc.scalar.activation(out=e2[:, 0:nb * H], in_=t[:, 0:nb * H],
                                         func=Act.Exp, scale=NEG)
                    nc.vector.tensor_tensor(out=e1[:, 0:nb * H], in0=e1[:, 0:nb * H],
                                            in1=e2[:, 0:nb * H], op=Alu.max)
                    exbf = epool.tile([P, 8 * H], dt.bfloat16, tag="exbf")
                    nc.vector.tensor_copy(out=exbf[:, 0:nb * H], in_=e1[:, 0:nb * H])

                    # scale: Gs = G(+ones) * ex
                    gs = gspool.tile([P, 8 * width], dt.bfloat16, tag="gs")
                    nc.vector.tensor_tensor(
                        out=gs[:, 0:nb * width].rearrange(
                            "p (b h y) -> p b h y", h=H, y=dg + 2),
                        in0=g[:, 0:nb * es].rearrange(
                            "p (b w) -> p b w", w=es)[:, :, 0:width].rearrange(
                            "p b (h y) -> p b h y", y=dg + 2),
                        in1=exbf[:, 0:nb * H].rearrange(
                            "p (b h) -> p b h", h=H).unsqueeze(3).to_broadcast(
                            [P, nb, H, dg + 2]),
                        op=Alu.mult)

                    # matmuls
                    for j in range(nb):
                        b = b0 + j
                        wi = block_meta[b][0]
                        if wi not in pw:
                            pw[wi] = pw_pool.tile([P, width], dt.float32,
                                                  space="PSUM", tag="pw",
                                                  name=f"pw{layer}_{wi}")
                        nc.tensor.matmul(
                            out=pw[wi][:], lhsT=masks.pop(b),
                            rhs=gs[:, j * width:(j + 1) * width],
                            start=(b == first_of_w[wi]), stop=(b == last_of_w[wi]))
                        if b == last_of_w[wi]:
                            window_end(layer, wi, pw.pop(wi), dg, width)

            def window_end(layer, wi, ps, dg, width):
                dn = wpool.tile([P, H], dt.float32, tag="dn")
                nc.vector.tensor_scalar(
                    out=dn[:],
                    in0=ps[:].rearrange("p (h y) -> p h y", y=dg + 2)[:, :, dg:dg + 1],
                    scalar1=1e-30, scalar2=None, op0=Alu.max)
                rc = wpool.tile([P, H], dt.float32, tag="rc")
                nc.vector.reciprocal(out=rc[:], in_=dn[:])
                if layer == 1:
                    hr = wpool.tile([P, CIN], dt.float32, tag="hr")
                    nc.vector.tensor_tensor(
                        out=hr[:].rearrange("p (h d) -> p h d", h=H),
                        in0=ps[:].rearrange("p (h y) -> p h y", y=dg + 2)[:, :, 0:dg],
                        in1=rc[:].unsqueeze(2).to_broadcast([P, H, dg]),
                        op=Alu.mult)
                    nc.vector.tensor_tensor(out=hr[:], in0=hr[:], in1=b1_sb[:],
                                            op=Alu.add)
                    h2t = wpool.tile([P, CIN], dt.float32, tag="h2t")
                    nc.vector.tensor_scalar(out=h2t[:], in0=hr[:], scalar1=NEG,
                                            scalar2=None, op0=Alu.mult)
                    nc.vector.tensor_tensor(out=hr[:], in0=hr[:], in1=h2t[:],
                                            op=Alu.max)
                    nc.vector.tensor_copy(out=hp_sb[:, wi * CIN:(wi + 1) * CIN],
                                          in_=hr[:])
                else:
                    nc.vector.tensor_scalar(out=rc[:], in0=rc[:], scalar1=0.25,
                                            scalar2=None, op0=Alu.mult)
                    tmp = wpool.tile([P, H * D2], dt.float32, tag="otmp")
                    nc.vector.tensor_tensor(
                        out=tmp[:].rearrange("p (h d) -> p h d", h=H),
                        in0=ps[:].rearrange("p (h y) -> p h y", y=dg + 2)[:, :, 0:dg],
                        in1=rc[:].unsqueeze(2).to_broadcast([P, H, dg]),
                        op=Alu.mult)
                    red = wpool.tile([P, D2], dt.float32, tag="red")
                    nc.vector.tensor_reduce(
                        out=red[:],
                        in_=tmp[:].rearrange("p (h d) -> p d h", h=H),
                        axis=mybir.AxisListType.X, op=Alu.add)
                    nc.vector.tensor_tensor(out=red[:], in0=red[:], in1=b2_sb[:],
                                            op=Alu.add)
                    nc.sync.dma_start(out=out_t[wi * P:(wi + 1) * P, :], in_=red[:])

            # ---------- program ----------
            node_matmul_phase(1)
            nc.gpsimd.collective_compute(
                "AllGather", mybir.AluOpType.bypass, replica_groups=rg,
                ins=[t1own[:, :]], outs=[T1[0:NCORE * NSHP, :]])
            edge_phase(1)
            node_matmul_phase(2)
            nc.gpsimd.collective_compute(
                "AllGather", mybir.AluOpType.bypass, replica_groups=rg,
                ins=[t2own[:, :]], outs=[T2[0:NCORE * NSHP, :]])
            edge_phase(2)

    nc.compile()
    return nc


def _host_inputs(inputs, sched):
    x = np.asarray(inputs['x'], np.float32)
    W1 = np.asarray(inputs['W1'], np.float32)
    W2 = np.asarray(inputs['W2'], np.float32)
    a_src1 = np.asarray(inputs['a_src1'], np.float32)
    a_dst1 = np.asarray(inputs['a_dst1'], np.float32)
    a_src2 = np.asarray(inputs['a_src2'], np.float32)
    a_dst2 = np.asarray(inputs['a_dst2'], np.float32)
    b1 = np.asarray(inputs['b1'], np.float32)
    b2 = np.asarray(inputs['b2'], np.float32)

    a1r = np.tile(np.concatenate([a_src1.reshape(-1), a_dst1.reshape(-1)])[None, :],
                  (P, 1)).astype(np.float32)
    a2r = np.tile(np.concatenate([a_src2.reshape(-1), a_dst2.reshape(-1)])[None, :],
                  (P, 1)).astype(np.float32)
    b1r = np.tile(b1[None, :], (P, 1)).astype(np.float32)
    b2r = np.tile(b2[None, :], (P, 1)).astype(np.float32)
    iota = np.tile(np.arange(P, dtype=np.float32)[None, :], (P, 1))
    ident = np.eye(P, dtype=np.float32).astype(bf16)

    in_maps = []
    for k in range(NCORE):
        xk = np.zeros((NSHP, CIN), np.float32)
        xk[:NSH] = x[k * NSH:(k + 1) * NSH]
        in_maps.append({
            "xT": np.ascontiguousarray(xk.T).astype(bf16),
            "W1b": W1.astype(bf16),
            "W2b": W2.astype(bf16),
            "a1r": a1r, "a2r": a2r, "b1r": b1r, "b2r": b2r,
            "iota_in": iota, "ident_in": ident,
            "sidx": sched['srcpk'][k],
            "drel": np.ascontiguousarray(sched['dstrel_pk'][k]),
        })
    return in_maps


def kernel(**inputs):
    import os
    from concourse.bass_utils import run_bass_kernel_spmd

    edge_index = np.asarray(inputs['edge_index'])
    sched = _prep(edge_index)
    nc = _build(sched)
    in_maps = _host_inputs(inputs, sched)

    trace = os.environ.get("KERNEL_TRACE") == "1"
    if trace:
        import profhook
        profhook.install()
    res = run_bass_kernel_spmd(nc, in_maps, core_ids=list(range(NCORE)),
                               trace=trace)
    if trace and res.exec_time_ns:
        print(f"HW exec time: {res.exec_time_ns} ns")
        kernel.exec_time_ns = res.exec_time_ns
        kernel.res = res

    out = np.zeros((N, D2), np.float32)
    for k in range(NCORE):
        out[k * NSH:(k + 1) * NSH] = res.results[k]["out"][:NSH]
    return out


# revision 15
# speedup vs baseline: 1.1760x; 1.1760x over previous
"""Two-layer GAT (PyG GATConv x2) on 8 Trainium2 NeuronCores via Bass.

Strategy (dst-sharded, graph-parallel):
- Nodes sharded 8 ways by destination range (6250/core, padded to 6272).
- Per layer: local feature matmul -> build a gather table row per node
  [G-per-head|1.0 ... | al_src | al_dst] in bf16 -> on-device AllGather ->
  edge phase: edges sorted by dst window (128 dsts), bulk dma_gather of
  source rows, softmax WITHOUT max-subtraction (exponents bounded, fp32-safe),
  segment-sums via one-hot "staircase" mask matmuls accumulating in PSUM.
  The softmax denominator rides in the same matmul through baked 1.0 columns.
  al_dst is broadcast dst->edges with a PE transpose of the mask + tiny matmul
  (avoids a second per-edge gather).
- leaky_relu inside exp via exp(leaky(t)) = max(exp(t), exp(0.2 t)).
"""
import math
import sys

import numpy as np
import ml_dtypes

sys.path.insert(0, '/opt/trn_rl_repo')

bf16 = ml_dtypes.bfloat16

P = 128
NCORE = 8
N = 50000
NSH = 6250
NSHP = 6272          # 49 * 128
NW = NSHP // P       # 49 windows
HALF = 4 * NSHP      # 25088 rows per half-table
CIN = 256
H = 4
D1 = 64
D2 = 32
R1 = 384                  # table-1 row stride (256B-mult; content 268)
R2 = 256                  # table-2 row stride (256B-mult; content 140)
ES1 = 384                 # gather elem count L1 (768B, covers 268 + spill)
ES2 = 256                 # gather elem count L2 (512B, covers 140 + spill)
TROWS = 2 * HALF + 256    # table alloc rows (incl. spill pad)
PADREL = 200.0            # dstrel sentinel for pad edges (kills mask column)
NEG = 0.2


def _prep(edge_index):
    """Host-side: shard + sort edges, build schedule and index arrays."""
    src = np.concatenate([edge_index[0], np.arange(N, dtype=np.int64)]).astype(np.int64)
    dst = np.concatenate([edge_index[1], np.arange(N, dtype=np.int64)]).astype(np.int64)
    owner = dst // NSH
    dloc = (dst - owner * NSH).astype(np.int32)
    srcpad = ((src // NSH) * NSHP + (src % NSH)).astype(np.int32)
    w = dloc // P
    drel = (dloc % P).astype(np.int32)
    half = (srcpad >= HALF).astype(np.int32)
    srcrel = np.where(half == 1, srcpad - HALF, srcpad).astype(np.int32)

    # per (core, window, half) edge lists
    counts = np.zeros((NCORE, NW, 2), np.int64)
    percore = []
    for k in range(NCORE):
        sel = np.nonzero(owner == k)[0]
        key = (w[sel] * 2 + half[sel]).astype(np.int64)
        order = np.argsort(key, kind='stable')
        sel = sel[order]
        kk = key[order]
        cnt = np.bincount(kk, minlength=NW * 2).reshape(NW, 2)
        counts[k] = cnt
        percore.append((srcrel[sel], drel[sel], cnt))

    nblk = np.maximum(1, np.ceil(counts.max(axis=0) / P).astype(np.int64))  # [NW,2]
    block_meta = []           # (window, half) per block
    for wi in range(NW):
        for h in range(2):
            block_meta += [(wi, h)] * int(nblk[wi, h])
    totblk = len(block_meta)

    # calls: runs of consecutive same-half blocks, <= 8 blocks each
    calls = []                # (b0, nb, half, col0)
    col = 0
    b = 0
    while b < totblk:
        h = block_meta[b][1]
        nb = 1
        while (b + nb < totblk and block_meta[b + nb][1] == h
               and nb < 8):
            nb += 1
        calls.append((b, nb, h, col))
        col += nb * 8
        b += nb
    ccols = col

    # per-core lane arrays
    srcidx = np.zeros((NCORE, totblk, P), np.int16)
    dstrel = np.full((NCORE, totblk, P), PADREL, np.float32)
    for k in range(NCORE):
        es, ed, cnt = percore[k]
        pos = 0
        blk = 0
        for wi in range(NW):
            for h in range(2):
                c = int(cnt[wi, h])
                nb = int(nblk[wi, h])
                lanes = np.arange(c)
                srcidx[k, blk + lanes // P, lanes % P] = es[pos:pos + c]
                dstrel[k, blk + lanes // P, lanes % P] = ed[pos:pos + c]
                pos += c
                blk += nb
        assert pos == len(es)

    # pack call indices: [128, ccols] int16 per core
    srcpk = np.zeros((NCORE, P, ccols), np.int16)
    for k in range(NCORE):
        for (b0, nb, h, c0) in calls:
            ni = nb * P
            flat = srcidx[k, b0:b0 + nb].reshape(ni)   # flat[j*128+p]
            wrap = flat.reshape(-1, 16).T              # [16, ni/16]
            srcpk[k, :, c0:c0 + ni // 16] = np.tile(wrap, (8, 1))
    dstrel_pk = np.ascontiguousarray(
        dstrel.transpose(0, 2, 1)).astype(np.float32)  # [NCORE, 128, totblk]

    first_of_w = {}
    last_of_w = {}
    for b, (wi, h) in enumerate(block_meta):
        if wi not in first_of_w:
            first_of_w[wi] = b
        last_of_w[wi] = b
    return dict(block_meta=block_meta, calls=calls, totblk=totblk, ccols=ccols,
                srcpk=srcpk, dstrel_pk=dstrel_pk,
                first_of_w=first_of_w, last_of_w=last_of_w)


def _build(sched):
    import concourse.bass as bass
    import concourse.tile as tile
    from concourse import bacc, mybir, library_config
    from concourse.bass import AP

    dt = mybir.dt
    Alu = mybir.AluOpType
    Act = mybir.ActivationFunctionType

    totblk = sched['totblk']
    ccols = sched['ccols']
    calls = sched['calls']
    block_meta = sched['block_meta']
    first_of_w = sched['first_of_w']
    last_of_w = sched['last_of_w']

    nc = bacc.Bacc("TRN2", target_bir_lowering=False, debug=False,
                   num_devices=NCORE, num_swdge_queues=4)

    # ---- I/O ----
    xT = nc.dram_tensor("xT", [CIN, NSHP], dt.bfloat16, kind="ExternalInput")
    W1 = nc.dram_tensor("W1b", [CIN, CIN], dt.bfloat16, kind="ExternalInput")
    W2 = nc.dram_tensor("W2b", [CIN, H * D2], dt.bfloat16, kind="ExternalInput")
    a1r = nc.dram_tensor("a1r", [P, 2 * CIN], dt.float32, kind="ExternalInput")
    a2r = nc.dram_tensor("a2r", [P, 2 * H * D2], dt.float32, kind="ExternalInput")
    b1r = nc.dram_tensor("b1r", [P, CIN], dt.float32, kind="ExternalInput")
    b2r = nc.dram_tensor("b2r", [P, D2], dt.float32, kind="ExternalInput")
    iota_in = nc.dram_tensor("iota_in", [P, P], dt.float32, kind="ExternalInput")
    ident_in = nc.dram_tensor("ident_in", [P, P], dt.bfloat16, kind="ExternalInput")
    sidx = nc.dram_tensor("sidx", [P, ccols], dt.int16, kind="ExternalInput")
    drel = nc.dram_tensor("drel", [P, totblk], dt.float32, kind="ExternalInput")
    out_t = nc.dram_tensor("out", [NSHP, D2], dt.float32, kind="ExternalOutput")

    # ---- internal DRAM ----
    t1own = nc.dram_tensor("t1own", [NSHP, R1], dt.bfloat16)
    t2own = nc.dram_tensor("t2own", [NSHP, R2], dt.bfloat16)
    T1 = nc.dram_tensor("T1", [TROWS, R1], dt.bfloat16, addr_space="Shared")
    T2 = nc.dram_tensor("T2", [TROWS, R2], dt.bfloat16, addr_space="Shared")

    rg = [list(range(NCORE))]

    with tile.TileContext(nc) as tc:
        import contextlib
        ctx = contextlib.ExitStack()
        with ctx:
            cpool = ctx.enter_context(tc.tile_pool(name="consts", bufs=1))
            gpool = ctx.enter_context(tc.tile_pool(name="g", bufs=10))
            gspool = ctx.enter_context(tc.tile_pool(name="gs", bufs=6))
            mpool = ctx.enter_context(tc.tile_pool(name="mask", bufs=4))
            spool = ctx.enter_context(tc.tile_pool(name="ssb", bufs=8))
            epool = ctx.enter_context(tc.tile_pool(name="ex", bufs=12))
            wpool = ctx.enter_context(tc.tile_pool(name="wend", bufs=6))
            rowpool = ctx.enter_context(tc.tile_pool(name="trow", bufs=4))
            xpool = ctx.enter_context(tc.tile_pool(name="xt", bufs=4))
            pw_pool = ctx.enter_context(tc.tile_pool(name="pw", bufs=2, space="PSUM"))
            tps_pool = ctx.enter_context(tc.tile_pool(name="tps", bufs=2, space="PSUM"))
            ad_pool = ctx.enter_context(tc.tile_pool(name="adp", bufs=2, space="PSUM"))
            hp_pool = ctx.enter_context(tc.tile_pool(name="hps", bufs=2, space="PSUM"))

            nc.gpsimd.load_library(library_config.mlp)

            # ---- persistent constants ----
            iota_sb = cpool.tile([P, P], dt.float32, tag="iota")
            nc.sync.dma_start(out=iota_sb[:], in_=iota_in[:, :])
            ident_sb = cpool.tile([P, P], dt.bfloat16, tag="ident")
            nc.sync.dma_start(out=ident_sb[:], in_=ident_in[:, :])
            w1a = cpool.tile([P, CIN], dt.bfloat16, tag="w1a")
            nc.sync.dma_start(out=w1a[:], in_=W1[0:P, :])
            w1b = cpool.tile([P, CIN], dt.bfloat16, tag="w1b")
            nc.sync.dma_start(out=w1b[:], in_=W1[P:2 * P, :])
            w2a = cpool.tile([P, H * D2], dt.bfloat16, tag="w2a")
            nc.sync.dma_start(out=w2a[:], in_=W2[0:P, :])
            w2b = cpool.tile([P, H * D2], dt.bfloat16, tag="w2b")
            nc.sync.dma_start(out=w2b[:], in_=W2[P:2 * P, :])
            a1_sb = cpool.tile([P, 2 * CIN], dt.float32, tag="a1")
            nc.sync.dma_start(out=a1_sb[:], in_=a1r[:, :])
            a2_sb = cpool.tile([P, 2 * H * D2], dt.float32, tag="a2")
            nc.sync.dma_start(out=a2_sb[:], in_=a2r[:, :])
            b1_sb = cpool.tile([P, CIN], dt.float32, tag="b1")
            nc.sync.dma_start(out=b1_sb[:], in_=b1r[:, :])
            b2_sb = cpool.tile([P, D2], dt.float32, tag="b2")
            nc.sync.dma_start(out=b2_sb[:], in_=b2r[:, :])
            sidx_sb = cpool.tile([P, ccols], dt.int16, tag="sidx")
            nc.sync.dma_start(out=sidx_sb[:], in_=sidx[:, :])
            drel_sb = cpool.tile([P, totblk], dt.float32, tag="drel")
            nc.sync.dma_start(out=drel_sb[:], in_=drel[:, :])
            alown1 = cpool.tile([P, NW * 2 * H], dt.bfloat16, tag="alo1")
            alown2 = cpool.tile([P, NW * 2 * H], dt.bfloat16, tag="alo2")
            hp_sb = cpool.tile([P, NW * CIN], dt.bfloat16, tag="hp")

            def node_matmul_phase(layer):
                """x@W (or h'@W2) per window; build table rows; als."""
                for wi in range(NW):
                    if layer == 1:
                        ps = hp_pool.tile([P, CIN], dt.float32, space="PSUM", tag="hps")
                        la = xpool.tile([P, P], dt.bfloat16, tag="xa")
                        nc.sync.dma_start(out=la[:], in_=xT[0:P, wi * P:(wi + 1) * P])
                        lb = xpool.tile([P, P], dt.bfloat16, tag="xb")
                        nc.sync.dma_start(out=lb[:], in_=xT[P:2 * P, wi * P:(wi + 1) * P])
                        nc.tensor.matmul(out=ps[:], lhsT=la[:], rhs=w1a[:],
                                         start=True, stop=False)
                        nc.tensor.matmul(out=ps[:], lhsT=lb[:], rhs=w1b[:],
                                         start=False, stop=True)
                        width, dg, rw, es, alo, a_sb = CIN, D1, R1, ES1, alown1, a1_sb
                        town = t1own
                    else:
                        # transpose h' window chunks -> lhsT
                        hsl = hp_sb[:, wi * CIN:(wi + 1) * CIN]
                        ps = hp_pool.tile([P, H * D2], dt.float32, space="PSUM", tag="hps")
                        for kc in range(2):
                            tp = tps_pool.tile([P, P], dt.bfloat16, space="PSUM", tag="tps")
                            nc.tensor.transpose(out=tp[:], in_=hsl[:, kc * P:(kc + 1) * P],
                                                identity=ident_sb[:])
                            hT = spool.tile([P, P], dt.bfloat16, tag="hT")
                            nc.scalar.copy(out=hT[:], in_=tp[:])
                            nc.tensor.matmul(out=ps[:], lhsT=hT[:],
                                             rhs=(w2a if kc == 0 else w2b)[:],
                                             start=(kc == 0), stop=(kc == 1))
                        width, dg, rw, es, alo, a_sb = H * D2, D2, R2, ES2, alown2, a2_sb
                        town = t2own

                    # als: tmp = ps(x2) * a_rep ; reduce per head
                    tmp = wpool.tile([P, 2 * width], dt.float32, tag="altmp")
                    nc.vector.tensor_tensor(
                        out=tmp[:], in0=ps[:].unsqueeze(1).to_broadcast([P, 2, width]),
                        in1=a_sb[:].rearrange("p (t x) -> p t x", t=2),
                        op=Alu.mult)
                    alof = wpool.tile([P, 8], dt.float32, tag="alof")
                    nc.vector.tensor_reduce(
                        out=alof[:],
                        in_=tmp[:].rearrange("p (t h d) -> p t h d", t=2, h=H),
                        axis=mybir.AxisListType.X, op=Alu.add)
                    nc.vector.tensor_copy(out=alo[:, wi * 8:wi * 8 + 8], in_=alof[:])

                    # table row
                    tr = rowpool.tile([P, rw], dt.bfloat16, tag="trow")
                    nc.gpsimd.memset(tr[:], 1.0)
                    nc.scalar.copy(
                        out=tr[:, 0:H * (dg + 2)].rearrange(
                            "p (h y) -> p h y", h=H)[:, :, 0:dg],
                        in_=ps[:].rearrange("p (h d) -> p h d", h=H))
                    nc.vector.tensor_copy(
                        out=tr[:, H * (dg + 2):H * (dg + 2) + 8], in_=alof[:])
                    nc.sync.dma_start(out=town[wi * P:(wi + 1) * P, :], in_=tr[:])

            def edge_phase(layer):
                T = T1 if layer == 1 else T2
                rw = R1 if layer == 1 else R2
                es = ES1 if layer == 1 else ES2
                dg = D1 if layer == 1 else D2
                width = H * (dg + 2)
                alo = alown1 if layer == 1 else alown2
                pw = {}
                masks = {}
                qn = 0
                for ci, (b0, nb, hf, c0) in enumerate(calls):
                    ni = nb * P
                    g = gpool.tile([P, 8 * es], dt.bfloat16, tag="g")
                    in_ap = AP(T, hf * HALF * rw, [[rw, TROWS - hf * HALF], [1, es]])
                    nc.gpsimd.dma_gather(
                        g[:, 0:nb * es].rearrange("p (b w) -> p b w", w=es),
                        in_ap, sidx_sb[:, c0:c0 + ni // 16], ni, ni, es,
                        elem_step=rw, queue_num=qn)
                    qn = (qn + 1) % 4

                    # masks (one grouped op per call) + Ad broadcast
                    adps = ad_pool.tile([P, 8 * H], dt.float32, space="PSUM", tag="adp")
                    mt = mpool.tile([P, 8 * P], dt.bfloat16, tag="m")
                    nc.vector.tensor_tensor(
                        out=mt[:, 0:nb * P].rearrange("p (b q) -> p b q", q=P),
                        in0=iota_sb[:, 0:P].unsqueeze(1).to_broadcast([P, nb, P]),
                        in1=drel_sb[:, b0:b0 + nb].unsqueeze(2).to_broadcast(
                            [P, nb, P]),
                        op=Alu.is_equal)
                    for j in range(nb):
                        b = b0 + j
                        wi = block_meta[b][0]
                        masks[b] = mt[:, j * P:(j + 1) * P]
                        tp = tps_pool.tile([P, P], dt.bfloat16, space="PSUM", tag="tps")
                        nc.tensor.transpose(out=tp[:], in_=masks[b],
                                            identity=ident_sb[:])
                        ssb = spool.tile([P, P], dt.bfloat16, tag="s")
                        nc.scalar.copy(out=ssb[:], in_=tp[:])
                        nc.tensor.matmul(
                            out=adps[:, j * H:(j + 1) * H], lhsT=ssb[:],
                            rhs=alo[:, wi * 8 + 4:wi * 8 + 8],
                            start=True, stop=True)

                    # ex chain (batched over nb blocks)
                    t = epool.tile([P, 8 * H], dt.float32, tag="t")
                    nc.vector.tensor_tensor(
                        out=t[:, 0:nb * H],
                        in0=g[:, 0:nb * es].rearrange(
                            "p (b w) -> p b w", w=es)[:, :, width:width + H],
                        in1=adps[:, 0:nb * H].rearrange("p (b h) -> p b h", h=H),
                        op=Alu.add)
                    e1 = epool.tile([P, 8 * H], dt.float32, tag="e1")
                    nc.scalar.activation(out=e1[:, 0:nb * H], in_=t[:, 0:nb * H],
                                         func=Act.Exp)
                    e2 = epool.tile([P, 8 * H], dt.float32, tag="e2")
                    nc.scalar.activation(out=e2[:, 0:nb * H], in_=t[:, 0:nb * H],
                                         func=Act.Exp, scale=NEG)
                    exbf = epool.tile([P, 8 * H], dt.bfloat16, tag="exbf")
                    nc.vector.tensor_tensor(out=exbf[:, 0:nb * H], in0=e1[:, 0:nb * H],
                                            in1=e2[:, 0:nb * H], op=Alu.max)

                    # scale: Gs = G(+ones) * ex
                    gs = gspool.tile([P, 8 * width], dt.bfloat16, tag="gs")
                    nc.vector.tensor_tensor(
                        out=gs[:, 0:nb * width].rearrange(
                            "p (b h y) -> p b h y", h=H, y=dg + 2),
                        in0=g[:, 0:nb * es].rearrange(
                            "p (b w) -> p b w", w=es)[:, :, 0:width].rearrange(
                            "p b (h y) -> p b h y", y=dg + 2),
                        in1=exbf[:, 0:nb * H].rearrange(
                            "p (b h) -> p b h", h=H).unsqueeze(3).to_broadcast(
                            [P, nb, H, dg + 2]),
                        op=Alu.mult)

                    # matmuls
                    for j in range(nb):
                        b = b0 + j
                        wi = block_meta[b][0]
                        if wi not in pw:
                            pw[wi] = pw_pool.tile([P, width], dt.float32,
                                                  space="PSUM", tag="pw",
                                                  name=f"pw{layer}_{wi}")
                        nc.tensor.matmul(
                            out=pw[wi][:], lhsT=masks.pop(b),
                            rhs=gs[:, j * width:(j + 1) * width],
                            start=(b == first_of_w[wi]), stop=(b == last_of_w[wi]))
                        if b == last_of_w[wi]:
                            window_end(layer, wi, pw.pop(wi), dg, width)

            def window_end(layer, wi, ps, dg, width):
                dn = wpool.tile([P, H], dt.float32, tag="dn")
                nc.vector.tensor_scalar(
                    out=dn[:],
                    in0=ps[:].rearrange("p (h y) -> p h y", y=dg + 2)[:, :, dg:dg + 1],
                    scalar1=1e-30, scalar2=None, op0=Alu.max)
                rc = wpool.tile([P, H], dt.float32, tag="rc")
                nc.vector.reciprocal(out=rc[:], in_=dn[:])
                if layer == 1:
                    hr = wpool.tile([P, CIN], dt.float32, tag="hr")
                    nc.vector.tensor_tensor(
                        out=hr[:].rearrange("p (h d) -> p h d", h=H),
                        in0=ps[:].rearrange("p (h y) -> p h y", y=dg + 2)[:, :, 0:dg],
                        in1=rc[:].unsqueeze(2).to_broadcast([P, H, dg]),
                        op=Alu.mult)
                    nc.vector.tensor_tensor(out=hr[:], in0=hr[:], in1=b1_sb[:],
                                            op=Alu.add)
                    h2t = wpool.tile([P, CIN], dt.float32, tag="h2t")
                    nc.vector.tensor_scalar(out=h2t[:], in0=hr[:], scalar1=NEG,
                                            scalar2=None, op0=Alu.mult)
                    nc.vector.tensor_tensor(
                        out=hp_sb[:, wi * CIN:(wi + 1) * CIN], in0=hr[:],
                        in1=h2t[:], op=Alu.max)
                else:
                    nc.vector.tensor_scalar(out=rc[:], in0=rc[:], scalar1=0.25,
                                            scalar2=None, op0=Alu.mult)
                    tmp = wpool.tile([P, H * D2], dt.float32, tag="otmp")
                    nc.vector.tensor_tensor(
                        out=tmp[:].rearrange("p (h d) -> p h d", h=H),
                        in0=ps[:].rearrange("p (h y) -> p h y", y=dg + 2)[:, :, 0:dg],
                        in1=rc[:].unsqueeze(2).to_broadcast([P, H, dg]),
                        op=Alu.mult)
                    red = wpool.tile([P, D2], dt.float32, tag="red")
                    nc.vector.tensor_reduce(
                        out=red[:],
                        in_=tmp[:].rearrange("p (h d) -> p d h", h=H),
                        axis=mybir.AxisListType.X, op=Alu.add)
                    nc.vector.tensor_tensor(out=red[:], in0=red[:], in1=b2_sb[:],
                                            op=Alu.add)
                    nc.sync.dma_start(out=out_t[wi * P:(wi + 1) * P, :], in_=red[:])

            # ---------- program ----------
            node_matmul_phase(1)
            nc.gpsimd.collective_compute(
                "AllGather", mybir.AluOpType.bypass, replica_groups=rg,
                ins=[t1own[:, :]], outs=[T1[0:NCORE * NSHP, :]])
            edge_phase(1)
            node_matmul_phase(2)
            nc.gpsimd.collective_compute(
                "AllGather", mybir.AluOpType.bypass, replica_groups=rg,
                ins=[t2own[:, :]], outs=[T2[0:NCORE * NSHP, :]])
            edge_phase(2)

    nc.compile()
    return nc


def _host_inputs(inputs, sched):
    x = np.asarray(inputs['x'], np.float32)
    W1 = np.asarray(inputs['W1'], np.float32)
    W2 = np.asarray(inputs['W2'], np.float32)
    a_src1 = np.asarray(inputs['a_src1'], np.float32)
    a_dst1 = np.asarray(inputs['a_dst1'], np.float32)
    a_src2 = np.asarray(inputs['a_src2'], np.float32)
    a_dst2 = np.asarray(inputs['a_dst2'], np.float32)
    b1 = np.asarray(inputs['b1'], np.float32)
    b2 = np.asarray(inputs['b2'], np.float32)

    a1r = np.tile(np.concatenate([a_src1.reshape(-1), a_dst1.reshape(-1)])[None, :],
                  (P, 1)).astype(np.float32)
    a2r = np.tile(np.concatenate([a_src2.reshape(-1), a_dst2.reshape(-1)])[None, :],
                  (P, 1)).astype(np.float32)
    b1r = np.tile(b1[None, :], (P, 1)).astype(np.float32)
    b2r = np.tile(b2[None, :], (P, 1)).astype(np.float32)
    iota = np.tile(np.arange(P, dtype=np.float32)[None, :], (P, 1))
    ident = np.eye(P, dtype=np.float32).astype(bf16)

    in_maps = []
    for k in range(NCORE):
        xk = np.zeros((NSHP, CIN), np.float32)
        xk[:NSH] = x[k * NSH:(k + 1) * NSH]
        in_maps.append({
            "xT": np.ascontiguousarray(xk.T).astype(bf16),
            "W1b": W1.astype(bf16),
            "W2b": W2.astype(bf16),
            "a1r": a1r, "a2r": a2r, "b1r": b1r, "b2r": b2r,
            "iota_in": iota, "ident_in": ident,
            "sidx": sched['srcpk'][k],
            "drel": np.ascontiguousarray(sched['dstrel_pk'][k]),
        })
    return in_maps


def kernel(**inputs):
    import os
    from concourse.bass_utils import run_bass_kernel_spmd

    edge_index = np.asarray(inputs['edge_index'])
    sched = _prep(edge_index)
    nc = _build(sched)
    in_maps = _host_inputs(inputs, sched)

    trace = os.environ.get("KERNEL_TRACE") == "1"
    if trace:
        import profhook
        profhook.install()
    res = run_bass_kernel_spmd(nc, in_maps, core_ids=list(range(NCORE)),
                               trace=trace)
    if trace and res.exec_time_ns:
        print(f"HW exec time: {res.exec_time_ns} ns")
        kernel.exec_time_ns = res.exec_time_ns
        kernel.res = res

    out = np.zeros((N, D2), np.float32)
    for k in range(NCORE):
        out[k * NSH:(k + 1) * NSH] = res.results[k]["out"][:NSH]
    return out
